# revision 15
# baseline (speedup 1.0000x reference)
"""Trainium2 Bass kernel for nn_FDC2_61108794688088.

Math: out[i, c] = BS * s1[i, c] + (W2 @ colsum)[c] + BS * b_fc[c]
  where s1 = z1 @ W_fc[:, :2048].T
        colsum = sum_j relu(z2f @ W_proj.T + b_proj)[j, :]
        W2 = W_fc[:, 2048:]

Sharding: data-parallel over batch across 8 cores. Each core computes
  - s1T_scaled = (BS * s1_shard).T            [65, 256]  (float32r matmul)
  - colsum_local [1024] of its 256 rows       (fp8 matmul, fp32 accum)
The only cross-core reduction is the [1024] colsum vector, done on host
during the gather step, along with the tiny [65] matvec against W2.

The projection matmul runs in fp8 E4M3 (weights pre-scaled by 64 so they
sit in the normal range; the 1/64 is folded into the relu's scale) with
DoubleRow packing: K is consumed 256 rows per matmul instruction.
"""

import os
import sys

import numpy as np


def _import_concourse():
    try:
        import concourse.bass  # noqa: F401
    except ImportError:
        for p in ("/opt/trn_rl_repo", "/root/.axon_site/_ro/trn_rl_repo"):
            if os.path.isdir(p) and p not in sys.path:
                sys.path.append(p)
        import concourse.bass  # noqa: F401


_import_concourse()

import ml_dtypes  # noqa: E402
from contextlib import ExitStack  # noqa: E402

import concourse.bacc as bacc  # noqa: E402
import concourse.tile as tile  # noqa: E402
from concourse.tile_rust import add_dep_helper  # noqa: E402
from concourse import mybir  # noqa: E402
from concourse import bass_utils  # noqa: E402

BS = 2048
HID = 2048
PIN = 3 * 56 * 56  # 9408
POUT = 1024
NCLS = 65
NCORES = 8
B = BS // NCORES  # 256 rows per core
KT2 = (PIN + 127) // 128  # 74 k-tiles for the projection (padded to 9472)
KP2 = KT2 // 2  # 37 DoubleRow k-pairs
KT1 = HID // 128  # 16 k-tiles for s1
MT = POUT // 128  # 8 m-tiles of output features
WSCALE = 64.0  # fp8 weight pre-scale

FP8 = ml_dtypes.float8_e4m3
N_Z2_CHUNKS = 4  # split the z2 load so PE can start early
N_WP_CHUNKS = 2

_NC_CACHE = None
LAST_RESULTS = None  # BassKernelResults of the most recent run (for profiling)


def _chunks(n, k):
    base, rem = divmod(n, k)
    out = []
    start = 0
    for i in range(k):
        size = base + (1 if i < rem else 0)
        out.append((start, size))
        start += size
    return out


def _build_nc():
    """Build the per-core Bass module (identical on all 8 cores)."""
    # Bacc (not raw Bass): its compile passes split multi-semaphore waits
    # into EventSemaphore instructions (TRN2 allows 1 wait per instruction).
    nc = bacc.Bacc(target_bir_lowering=False)
    dt = mybir.dt

    z2ft = nc.dram_tensor("z2ft", [128, KP2, 2, B], dt.float8e4, kind="ExternalInput")
    wpt = nc.dram_tensor(
        "wpt", [MT, 128, KP2, 2, 128], dt.float8e4, kind="ExternalInput"
    )
    bp = nc.dram_tensor("bp", [128, MT], dt.float32, kind="ExternalInput")
    # z1^T shard and 2048*W_fc[:, :2048]^T fused into one tensor so the first
    # float32r matmul (self-loading, single sync-wait slot) waits on one DMA.
    zw = nc.dram_tensor("zw", [128, KT1, B + NCLS], dt.float32r, kind="ExternalInput")

    s1t_out = nc.dram_tensor("s1t", [NCLS, B], dt.float32, kind="ExternalOutput")
    colsum_out = nc.dram_tensor("colsum", [128, MT], dt.float32, kind="ExternalOutput")

    with tile.TileContext(nc) as tc, ExitStack() as ctx:
        singles = ctx.enter_context(tc.tile_pool(name="singles", bufs=1))
        wp_pool = ctx.enter_context(tc.tile_pool(name="wp", bufs=3))
        ps_pool = ctx.enter_context(tc.tile_pool(name="ps", bufs=2, space="PSUM"))
        ps1_pool = ctx.enter_context(tc.tile_pool(name="ps1", bufs=1, space="PSUM"))
        relu_pool = ctx.enter_context(tc.tile_pool(name="relu", bufs=2))
        out_pool = ctx.enter_context(tc.tile_pool(name="outs", bufs=1))

        bp_sb = singles.tile([128, MT], dt.float32)
        nc.scalar.dma_start(out=bp_sb, in_=bp[:])

        # z2 arrives in chunks so m-tile 0's matmuls can start early
        # (small first chunk, large later ones to keep DMA packets big).
        z2_sb = singles.tile([128, KP2, 2, B], dt.float8e4)
        z2_splits = [(0, 8), (8, 14), (22, 15)]
        for start, size in z2_splits:
            nc.sync.dma_start(
                out=z2_sb[:, start : start + size],
                in_=z2ft[:, start : start + size],
            )

        zw_sb = singles.tile([128, KT1, B + NCLS], dt.float32r)
        nc.scalar.dma_start(out=zw_sb, in_=zw[:])

        colsum_sb = out_pool.tile([128, MT], dt.float32)

        # projection branch: for each 128-wide block of output features,
        # psum[m, n] = sum_K 64*W_proj[m, K] * z2f[n, K]  (DoubleRow fp8),
        # then relu(psum/64 + b) and row-sum over the local batch.
        # wp DMAs alternate between the sync and gpsimd queues so SDMA
        # engines interleave two descriptor streams.
        last_proj_mm = None
        for t in range(MT):
            wp_sb = wp_pool.tile([128, KP2, 2, 128], dt.float8e4, tag="wp")
            dma_eng = nc.gpsimd if t % 2 else nc.sync
            if t == 0:
                for start, size in [(0, 8), (8, 29)]:
                    dma_eng.dma_start(
                        out=wp_sb[:, start : start + size],
                        in_=wpt[t, :, start : start + size],
                    )
            else:
                dma_eng.dma_start(out=wp_sb, in_=wpt[t])
            ps = ps_pool.tile([128, B], dt.float32, tag="ps")
            for kp in range(KP2):
                last_proj_mm = nc.tensor.matmul(
                    ps,
                    lhsT=wp_sb[:, kp],
                    rhs=z2_sb[:, kp],
                    start=(kp == 0),
                    stop=(kp == KP2 - 1),
                    perf_mode=mybir.MatmulPerfMode.DoubleRow,
                )
            relu_sb = relu_pool.tile([128, B], dt.float32, tag="relu")
            nc.scalar.activation(
                out=relu_sb,
                in_=ps,
                func=mybir.ActivationFunctionType.Relu,
                bias=bp_sb[:, t : t + 1],
                scale=1.0 / WSCALE,
                accum_out=colsum_sb[:, t : t + 1],
            )
        nc.sync.dma_start(out=colsum_out[:], in_=colsum_sb)

        # s1 branch: psum[c, n] = sum_K 2048*W_fc[c, K] * z1[n, K] (K-tiled).
        # Ordered after the projection matmuls: the PE stream is in-order, so
        # putting these first would stall everything on the zw DMA.
        ps1 = ps1_pool.tile([NCLS, B], dt.float32, tag="ps1")
        for ki in range(KT1):
            mm = nc.tensor.matmul(
                ps1,
                lhsT=zw_sb[:, ki, B:],
                rhs=zw_sb[:, ki, :B],
                start=(ki == 0),
                stop=(ki == KT1 - 1),
            )
            if ki == 0:
                add_dep_helper(
                    mm.ins, last_proj_mm.ins, reason="keep s1 after projection"
                )
        s1_sb = out_pool.tile([NCLS, B], dt.float32)
        nc.vector.tensor_copy(out=s1_sb, in_=ps1)
        nc.scalar.dma_start(out=s1t_out[:], in_=s1_sb)

    if not nc.is_finalized():
        nc.finalize()
    return nc


def _prep_inputs(z1, z2, W_proj, b_proj, W_fc):
    """Host-side sharding + layout. Returns per-core input maps."""
    z2f = np.ascontiguousarray(z2.reshape(BS, PIN))

    # z2f^T padded to [74*128, 2048] fp8, per-core [128, 37, 2, 256]:
    # z2ft[p, t, j, n] = z2f^T[(2t+j)*128 + p, n]
    Z = np.zeros((KT2 * 128, BS), dtype=FP8)
    Z[:PIN] = z2f.T.astype(FP8)

    # 64 * W_proj^T padded, arranged [8, 128, 37, 2, 128]:
    # wpt[t, p, k, j, m] = 64*W_proj[t*128+m, (2k+j)*128+p]
    Wp = np.zeros((KT2 * 128, POUT), dtype=FP8)
    Wp[:PIN] = (W_proj.T * np.float32(WSCALE)).astype(FP8)
    wpt_host = np.ascontiguousarray(
        Wp.reshape(KP2, 2, 128, MT, 128).transpose(3, 2, 0, 1, 4)
    )

    bp_host = np.ascontiguousarray(b_proj.reshape(MT, 128).T).astype(np.float32)

    # 2048 * W_fc[:, :HID]^T arranged [128, 16, 65]
    w1t_host = np.ascontiguousarray(
        (np.float32(BS) * W_fc[:, :HID].T.astype(np.float32))
        .reshape(KT1, 128, NCLS)
        .transpose(1, 0, 2)
    ).astype(np.float32)

    in_maps = []
    for c in range(NCORES):
        sl = slice(c * B, (c + 1) * B)
        z2_shard = np.ascontiguousarray(
            Z[:, sl].reshape(KP2, 2, 128, B).transpose(2, 0, 1, 3)
        )
        z1_shard = (
            z1[sl].T.reshape(KT1, 128, B).transpose(1, 0, 2).astype(np.float32)
        )
        zw_shard = np.ascontiguousarray(
            np.concatenate([z1_shard, w1t_host], axis=2)
        )
        in_maps.append(
            {
                "z2ft": z2_shard,
                "wpt": wpt_host,
                "bp": bp_host,
                "zw": zw_shard,
            }
        )
    return in_maps


def kernel(z1, z2, W_proj, b_proj, W_fc, b_fc):
    global _NC_CACHE, LAST_RESULTS

    z1 = np.asarray(z1, dtype=np.float32)
    z2 = np.asarray(z2, dtype=np.float32)
    W_proj = np.asarray(W_proj, dtype=np.float32)
    b_proj = np.asarray(b_proj, dtype=np.float32)
    W_fc = np.asarray(W_fc, dtype=np.float32)
    b_fc = np.asarray(b_fc, dtype=np.float32)

    if _NC_CACHE is None:
        _NC_CACHE = _build_nc()
    nc = _NC_CACHE

    in_maps = _prep_inputs(z1, z2, W_proj, b_proj, W_fc)
    res = bass_utils.run_bass_kernel_spmd(nc, in_maps, core_ids=list(range(NCORES)))
    LAST_RESULTS = res

    # gather: concat s1T shards, sum colsum shards, add the broadcast vector
    A = np.concatenate([np.asarray(r["s1t"]).T for r in res.results], axis=0)
    colsum = np.zeros(POUT, dtype=np.float64)
    for r in res.results:
        colsum += np.asarray(r["colsum"]).T.reshape(POUT).astype(np.float64)
    vec = W_fc[:, HID:].astype(np.float64) @ colsum + np.float64(BS) * b_fc.astype(
        np.float64
    )
    out = A.astype(np.float64) + vec[None, :]
    return out.astype(np.float32)


# revision 16
# speedup vs baseline: 1.0805x; 1.0805x over previous
"""Trainium2 Bass kernel for nn_FDC2_61108794688088.

Math: out[i, c] = BS * s1[i, c] + (W2 @ colsum)[c] + BS * b_fc[c]
  where s1 = z1 @ W_fc[:, :2048].T
        colsum = sum_j relu(z2f @ W_proj.T + b_proj)[j, :]
        W2 = W_fc[:, 2048:]

Sharding: data-parallel over batch across 8 cores. Each core computes
  - s1T_scaled = (BS * s1_shard).T            [65, 256]  (float32r matmul)
  - colsum_local [1024] of its 256 rows       (fp8 matmul, fp32 accum)
The only cross-core reduction is the [1024] colsum vector, done on host
during the gather step, along with the tiny [65] matvec against W2.

The projection matmul runs in fp8 E4M3 (weights pre-scaled by 64 so they
sit in the normal range; the 1/64 is folded into the relu's scale) with
DoubleRow packing: K is consumed 256 rows per matmul instruction.
"""

import os
import sys

import numpy as np


def _import_concourse():
    try:
        import concourse.bass  # noqa: F401
    except ImportError:
        for p in ("/opt/trn_rl_repo", "/root/.axon_site/_ro/trn_rl_repo"):
            if os.path.isdir(p) and p not in sys.path:
                sys.path.append(p)
        import concourse.bass  # noqa: F401


_import_concourse()

import ml_dtypes  # noqa: E402
from contextlib import ExitStack  # noqa: E402

import concourse.bacc as bacc  # noqa: E402
import concourse.tile as tile  # noqa: E402
from concourse.tile_rust import add_dep_helper  # noqa: E402
from concourse import mybir  # noqa: E402
from concourse import bass_utils  # noqa: E402

BS = 2048
HID = 2048
PIN = 3 * 56 * 56  # 9408
POUT = 1024
NCLS = 65
NCORES = 8
B = BS // NCORES  # 256 rows per core
KT2 = (PIN + 127) // 128  # 74 k-tiles for the projection (padded to 9472)
KP2 = KT2 // 2  # 37 DoubleRow k-pairs
KT1 = HID // 128  # 16 k-tiles for s1
MT = POUT // 128  # 8 m-tiles of output features
WSCALE = 64.0  # fp8 weight pre-scale

FP8 = ml_dtypes.float8_e4m3
N_Z2_CHUNKS = 4  # split the z2 load so PE can start early
N_WP_CHUNKS = 2

_NC_CACHE = None
LAST_RESULTS = None  # BassKernelResults of the most recent run (for profiling)


def _chunks(n, k):
    base, rem = divmod(n, k)
    out = []
    start = 0
    for i in range(k):
        size = base + (1 if i < rem else 0)
        out.append((start, size))
        start += size
    return out


def _build_nc():
    """Build the per-core Bass module (identical on all 8 cores)."""
    # Bacc (not raw Bass): its compile passes split multi-semaphore waits
    # into EventSemaphore instructions (TRN2 allows 1 wait per instruction).
    nc = bacc.Bacc(target_bir_lowering=False)
    dt = mybir.dt

    z2ft = nc.dram_tensor("z2ft", [128, KP2, 2, B], dt.float8e4, kind="ExternalInput")
    wpt = nc.dram_tensor(
        "wpt", [MT, 128, KP2, 2, 128], dt.float8e4, kind="ExternalInput"
    )
    bp = nc.dram_tensor("bp", [128, MT], dt.float32, kind="ExternalInput")
    # z1^T shard and 2048*W_fc[:, :2048]^T fused into one tensor so the first
    # float32r matmul (self-loading, single sync-wait slot) waits on one DMA.
    zw = nc.dram_tensor("zw", [128, KT1, B + NCLS], dt.float32r, kind="ExternalInput")

    s1t_out = nc.dram_tensor("s1t", [NCLS, B], dt.float32, kind="ExternalOutput")
    colsum_out = nc.dram_tensor("colsum", [128, MT], dt.float32, kind="ExternalOutput")

    with tile.TileContext(nc) as tc, ExitStack() as ctx:
        singles = ctx.enter_context(tc.tile_pool(name="singles", bufs=1))
        wp_pool = ctx.enter_context(tc.tile_pool(name="wp", bufs=3))
        ps_pool = ctx.enter_context(tc.tile_pool(name="ps", bufs=2, space="PSUM"))
        ps1_pool = ctx.enter_context(tc.tile_pool(name="ps1", bufs=1, space="PSUM"))
        relu_pool = ctx.enter_context(tc.tile_pool(name="relu", bufs=2))
        out_pool = ctx.enter_context(tc.tile_pool(name="outs", bufs=1))

        # Queue layout (per-queue bandwidth is fair-shared and FIFO within a
        # queue): sync carries z2's first small chunk then the whole wp
        # stream (the PE-gating path), gpsimd carries the rest of z2,
        # scalar carries bp + zw (only needed mid-kernel for s1).
        z2_sb = singles.tile([128, KP2, 2, B], dt.float8e4)
        nc.sync.dma_start(out=z2_sb[:, 0:4], in_=z2ft[:, 0:4])
        nc.gpsimd.dma_start(out=z2_sb[:, 4:20], in_=z2ft[:, 4:20])
        nc.gpsimd.dma_start(out=z2_sb[:, 20:KP2], in_=z2ft[:, 20:KP2])

        bp_sb = singles.tile([128, MT], dt.float32)
        nc.scalar.dma_start(out=bp_sb, in_=bp[:])
        zw_sb = singles.tile([128, KT1, B + NCLS], dt.float32r)
        nc.scalar.dma_start(out=zw_sb, in_=zw[:])

        colsum_sb = out_pool.tile([128, MT], dt.float32)

        # projection branch: for each 128-wide block of output features,
        # psum[m, n] = sum_K 64*W_proj[m, K] * z2f[n, K]  (DoubleRow fp8),
        # then relu(psum/64 + b) and row-sum over the local batch.
        proj_mms = []
        for t in range(MT):
            wp_sb = wp_pool.tile([128, KP2, 2, 128], dt.float8e4, tag="wp")
            if t == 0:
                nc.sync.dma_start(out=wp_sb[:, 0:4], in_=wpt[t, :, 0:4])
                nc.sync.dma_start(out=wp_sb[:, 4:KP2], in_=wpt[t, :, 4:KP2])
            else:
                nc.sync.dma_start(out=wp_sb, in_=wpt[t])
            ps = ps_pool.tile([128, B], dt.float32, tag="ps")
            for kp in range(KP2):
                mm = nc.tensor.matmul(
                    ps,
                    lhsT=wp_sb[:, kp],
                    rhs=z2_sb[:, kp],
                    start=(kp == 0),
                    stop=(kp == KP2 - 1),
                    perf_mode=mybir.MatmulPerfMode.DoubleRow,
                )
                proj_mms.append(mm)
            relu_sb = relu_pool.tile([128, B], dt.float32, tag="relu")
            nc.scalar.activation(
                out=relu_sb,
                in_=ps,
                func=mybir.ActivationFunctionType.Relu,
                bias=bp_sb[:, t : t + 1],
                scale=1.0 / WSCALE,
                accum_out=colsum_sb[:, t : t + 1],
            )
        nc.sync.dma_start(out=colsum_out[:], in_=colsum_sb)

        # s1 branch: psum[c, n] = sum_K 2048*W_fc[c, K] * z1[n, K] (K-tiled).
        # The PE stream is in-order; slot these after m-tile 3 — by then the
        # zw DMA has drained, and the PE is starving for wp anyway.
        anchor_mm = proj_mms[4 * KP2 - 1]
        ps1 = ps1_pool.tile([NCLS, B], dt.float32, tag="ps1")
        for ki in range(KT1):
            mm = nc.tensor.matmul(
                ps1,
                lhsT=zw_sb[:, ki, B:],
                rhs=zw_sb[:, ki, :B],
                start=(ki == 0),
                stop=(ki == KT1 - 1),
            )
            if ki == 0:
                add_dep_helper(
                    mm.ins, anchor_mm.ins, reason="s1 after projection m-tile 3"
                )
        add_dep_helper(
            proj_mms[4 * KP2].ins, mm.ins, reason="m-tile 4 after s1"
        )
        s1_sb = out_pool.tile([NCLS, B], dt.float32)
        nc.vector.tensor_copy(out=s1_sb, in_=ps1)
        nc.scalar.dma_start(out=s1t_out[:], in_=s1_sb)

    if not nc.is_finalized():
        nc.finalize()
    return nc


def _prep_inputs(z1, z2, W_proj, b_proj, W_fc):
    """Host-side sharding + layout. Returns per-core input maps."""
    z2f = np.ascontiguousarray(z2.reshape(BS, PIN))

    # z2f^T padded to [74*128, 2048] fp8, per-core [128, 37, 2, 256]:
    # z2ft[p, t, j, n] = z2f^T[(2t+j)*128 + p, n]
    Z = np.zeros((KT2 * 128, BS), dtype=FP8)
    Z[:PIN] = z2f.T.astype(FP8)

    # 64 * W_proj^T padded, arranged [8, 128, 37, 2, 128]:
    # wpt[t, p, k, j, m] = 64*W_proj[t*128+m, (2k+j)*128+p]
    Wp = np.zeros((KT2 * 128, POUT), dtype=FP8)
    Wp[:PIN] = (W_proj.T * np.float32(WSCALE)).astype(FP8)
    wpt_host = np.ascontiguousarray(
        Wp.reshape(KP2, 2, 128, MT, 128).transpose(3, 2, 0, 1, 4)
    )

    bp_host = np.ascontiguousarray(b_proj.reshape(MT, 128).T).astype(np.float32)

    # 2048 * W_fc[:, :HID]^T arranged [128, 16, 65]
    w1t_host = np.ascontiguousarray(
        (np.float32(BS) * W_fc[:, :HID].T.astype(np.float32))
        .reshape(KT1, 128, NCLS)
        .transpose(1, 0, 2)
    ).astype(np.float32)

    in_maps = []
    for c in range(NCORES):
        sl = slice(c * B, (c + 1) * B)
        z2_shard = np.ascontiguousarray(
            Z[:, sl].reshape(KP2, 2, 128, B).transpose(2, 0, 1, 3)
        )
        z1_shard = (
            z1[sl].T.reshape(KT1, 128, B).transpose(1, 0, 2).astype(np.float32)
        )
        zw_shard = np.ascontiguousarray(
            np.concatenate([z1_shard, w1t_host], axis=2)
        )
        in_maps.append(
            {
                "z2ft": z2_shard,
                "wpt": wpt_host,
                "bp": bp_host,
                "zw": zw_shard,
            }
        )
    return in_maps


def kernel(z1, z2, W_proj, b_proj, W_fc, b_fc):
    global _NC_CACHE, LAST_RESULTS

    z1 = np.asarray(z1, dtype=np.float32)
    z2 = np.asarray(z2, dtype=np.float32)
    W_proj = np.asarray(W_proj, dtype=np.float32)
    b_proj = np.asarray(b_proj, dtype=np.float32)
    W_fc = np.asarray(W_fc, dtype=np.float32)
    b_fc = np.asarray(b_fc, dtype=np.float32)

    if _NC_CACHE is None:
        _NC_CACHE = _build_nc()
    nc = _NC_CACHE

    in_maps = _prep_inputs(z1, z2, W_proj, b_proj, W_fc)
    res = bass_utils.run_bass_kernel_spmd(nc, in_maps, core_ids=list(range(NCORES)))
    LAST_RESULTS = res

    # gather: concat s1T shards, sum colsum shards, add the broadcast vector
    A = np.concatenate([np.asarray(r["s1t"]).T for r in res.results], axis=0)
    colsum = np.zeros(POUT, dtype=np.float64)
    for r in res.results:
        colsum += np.asarray(r["colsum"]).T.reshape(POUT).astype(np.float64)
    vec = W_fc[:, HID:].astype(np.float64) @ colsum + np.float64(BS) * b_fc.astype(
        np.float64
    )
    out = A.astype(np.float64) + vec[None, :]
    return out.astype(np.float32)
